# revision 15
# baseline (speedup 1.0000x reference)
"""Trainium2 Bass kernel for sparse attention with relation bias.

Computes, for inputs (B=4, N=512, C=128, H=8, HS=16):
  qkv = joint @ W_qkv^T -> q,k,v
  attn = softmax((q k^T + rel @ W_r^T) * conn * HS^-0.5)
  out  = (attn @ v) @ W_proj^T + b_proj

Sharding: 8 cores, core i handles batch b=i//2 and n-row half i%2 (256 rows).
No collectives — each core computes its own output rows; host gathers.

Layout: packed attention rows. For each group of G=16 n-rows, logits live in
ONE [128, 512] tile with partition p = j*8 + h (j = n-row within group,
h = head). This is possible because:
- attn_J uses a block-"diagonal" packed-Q lhsT (Qpacked[c, j*8+h] = q[n_j,h,d]
  for c = h*16+d, zero elsewhere) against the natural kT — one matmul/group.
- the relation bias per n-row j uses a [128,128] lhsT WRJ_j that is zero
  except columns j*8+h = W_r[h, :] — accumulating zeros into the other
  partitions is free, and the output AP stays 128-partition base-0 aligned.

rel is pre-transposed and pre-cast to bf16 on the host ([C, rows] layout), so
the kernel streams it with plain HWDGE DMA and feeds matmuls directly — no
on-chip transposes, and half the HBM bytes of the f32 layout.
"""

import sys

sys.path.insert(0, "/opt/trn_rl_repo")

import numpy as np
import ml_dtypes

import concourse.bass as bass
import concourse.tile as tile
from concourse import bacc, mybir
from concourse.masks import make_identity
from contextlib import ExitStack

F32 = mybir.dt.float32
BF16 = mybir.dt.bfloat16
FP8 = mybir.dt.float8e4
U32 = mybir.dt.uint32

# Problem constants (hardcoded per spec)
B, N, C, H = 4, 512, 128, 8
HS = C // H  # 16
SCALE = float(HS) ** -0.5
NCORES = 8
P = 128  # partitions
MC = N // P  # m-chunks per row = 4
G = 16  # n-rows per group
GH = 8  # n-rows per rel DMA half-load


def build_graph(NH):
    """Build the SPMD single-core graph. NH = n-rows per core."""
    NG = NH // G  # groups

    nc = bacc.Bacc("TRN2", target_bir_lowering=False, debug=False)
    rel_d = nc.declare_dram_parameter("relT", [C, NH * N], FP8, isOutput=False)
    conn_d = nc.declare_dram_parameter("conn", [NH, N], BF16, isOutput=False)
    jT_d = nc.declare_dram_parameter("jointT", [C, N], BF16, isOutput=False)
    jTq_d = nc.declare_dram_parameter("jointTq", [C, NH], BF16, isOutput=False)
    wqkvT_d = nc.declare_dram_parameter("wqkvT", [C, 3 * C], BF16, isOutput=False)
    wrT_d = nc.declare_dram_parameter("wrT", [C, H], BF16, isOutput=False)
    wpT_d = nc.declare_dram_parameter("wprojT", [C, C], BF16, isOutput=False)
    bp_d = nc.declare_dram_parameter("bproj", [1, C], BF16, isOutput=False)
    out_d = nc.declare_dram_parameter("out", [NH, C], F32, isOutput=True)

    with tile.TileContext(nc) as tc, ExitStack() as ctx:
        singles = ctx.enter_context(tc.tile_pool(name="singles", bufs=1))
        relp = ctx.enter_context(tc.tile_pool(name="relp", bufs=6))
        connp = ctx.enter_context(tc.tile_pool(name="connp", bufs=3))
        qpackp = ctx.enter_context(tc.tile_pool(name="qpackp", bufs=2))
        logitp = ctx.enter_context(tc.tile_pool(name="logitp", bufs=2))
        attnwp = ctx.enter_context(tc.tile_pool(name="attnwp", bufs=3))
        aTp = ctx.enter_context(tc.tile_pool(name="aTp", bufs=3))
        xsbp = ctx.enter_context(tc.tile_pool(name="xsbp", bufs=2))
        smallp = ctx.enter_context(tc.tile_pool(name="smallp", bufs=8))
        outp = ctx.enter_context(tc.tile_pool(name="outp", bufs=3))

        ps_attn = ctx.enter_context(tc.tile_pool(name="ps_attn", bufs=3, space="PSUM"))
        ps_tp = ctx.enter_context(tc.tile_pool(name="ps_tp", bufs=1, space="PSUM"))
        ps_x = ctx.enter_context(tc.tile_pool(name="ps_x", bufs=1, space="PSUM"))
        ps_o = ctx.enter_context(tc.tile_pool(name="ps_o", bufs=1, space="PSUM"))
        ps_conn = ctx.enter_context(tc.tile_pool(name="ps_conn", bufs=2, space="PSUM"))

        # ---- constants / weights ----
        ident = singles.tile([P, P], BF16)
        make_identity(nc, ident)
        wqkvT = singles.tile([P, 3 * C], BF16)
        nc.sync.dma_start(out=wqkvT, in_=wqkvT_d[:, :])
        wrT = singles.tile([P, H], BF16)
        nc.sync.dma_start(out=wrT, in_=wrT_d[:, :])
        wpT = singles.tile([P, C], BF16)
        nc.sync.dma_start(out=wpT, in_=wpT_d[:, :])
        bp = singles.tile([1, C], BF16)
        nc.sync.dma_start(out=bp, in_=bp_d[:, :])
        ones16 = singles.tile([1, G], BF16)
        nc.vector.memset(ones16, 1.0)
        jT = singles.tile([P, N], BF16)
        nc.sync.dma_start(out=jT, in_=jT_d[:, :])
        jTq = singles.tile([P, NH], BF16)
        nc.sync.dma_start(out=jTq, in_=jTq_d[:, :])

        # Mask[c, j*8+h] = 1.0 iff c//16 == h  (16 <= iota < 16 window on
        # iota = c - 16*h; j enters the pattern with step 0)
        mask = singles.tile([P, P], BF16)
        nc.gpsimd.memset(mask, 1.0)
        nc.gpsimd.affine_select(
            out=mask, in_=mask, compare_op=mybir.AluOpType.is_ge, fill=0.0,
            base=0, pattern=[[0, G], [-HS, H]], channel_multiplier=1,
        )
        nc.gpsimd.affine_select(
            out=mask, in_=mask, compare_op=mybir.AluOpType.is_ge, fill=0.0,
            base=HS - 1, pattern=[[0, G], [HS, H]], channel_multiplier=-1,
        )

        # sel16[j, j*8+h] = 1.0 — broadcasts conn rows x8 via the PE.
        sel16 = singles.tile([G, P], BF16)
        nc.gpsimd.memset(sel16, 1.0)
        nc.gpsimd.affine_select(
            out=sel16, in_=sel16, compare_op=mybir.AluOpType.is_ge, fill=0.0,
            base=0, pattern=[[1, P]], channel_multiplier=-H,
        )
        nc.gpsimd.affine_select(
            out=sel16, in_=sel16, compare_op=mybir.AluOpType.is_ge, fill=0.0,
            base=H - 1, pattern=[[-1, P]], channel_multiplier=H,
        )

        # wrj4[c, r, (j%4)*8+h] = W_r[h, c] for r == j%4; zero elsewhere.
        # Used as a [128, 32] lhsT so n-row j's bias lands in the 32-partition
        # strip j//4 at offset (j%4)*8 — four strips run col-tiled in parallel.
        wrj4 = singles.tile([P, 4, 32], BF16)
        nc.vector.memset(wrj4, 0.0)
        for r in range(4):
            nc.vector.tensor_copy(wrj4[:, r, r * H : (r + 1) * H], wrT)

        # ---- prep: natural-layout qkv projections ----
        kT = singles.tile([P, N], BF16)
        vnat = singles.tile([P, MC, P], BF16)
        qTq = singles.tile([P, NH], BF16)

        pk = ps_attn.tile([P, N], F32, tag="attn")
        nc.tensor.matmul(pk, lhsT=wqkvT[:, C : 2 * C], rhs=jT, start=True, stop=True)
        nc.vector.tensor_copy(kT, pk)
        pq = ps_attn.tile([P, N], F32, tag="attn")
        nc.tensor.matmul(pq[:, :NH], lhsT=wqkvT[:, :C], rhs=jTq, start=True, stop=True)
        nc.vector.tensor_copy(qTq, pq[:, :NH])
        for t in range(MC):
            pv = ps_attn.tile([P, N], F32, tag="attn")
            nc.tensor.matmul(
                pv[:, :P], lhsT=jT[:, t * P : (t + 1) * P],
                rhs=wqkvT[:, 2 * C :], start=True, stop=True,
            )
            nc.vector.tensor_copy(vnat[:, t, :], pv[:, :P])

        # Software pipeline, shifted by one group: in iteration g the PE runs
        # group g's J+rel matmuls while DVE/ACT finish group g-1's softmax and
        # the PE then does g-1's transposes/attn@v/proj. The PE queue never
        # waits on a same-group DVE/ACT result, so it stays busy (HAM warm).
        prev = None  # attn_w of group g-1

        def phase_head(g):
            # one 1MB fp8 load per group: [128, 8192], 8KB contiguous/partition
            relg = relp.tile([P, G * N], FP8, tag="rel")
            nc.sync.dma_start(out=relg, in_=rel_d[:, g * G * N : (g + 1) * G * N])

            # conn_e[j*8+h, m] = conn[g*16+j, m]: load raw rows once, replicate
            # x8 with a selector matmul on the PE, evacuate to SBUF bf16.
            conn_g = connp.tile([G, N], BF16)
            nc.sync.dma_start(out=conn_g, in_=conn_d[g * G : (g + 1) * G, :])
            conn_ps = ps_conn.tile([P, N], F32, tag="conn")
            nc.tensor.matmul(conn_ps, lhsT=sel16, rhs=conn_g, start=True, stop=True)
            conn_e = connp.tile([P, N], BF16)
            if g % 2 == 0:
                nc.vector.tensor_copy(conn_e, conn_ps)
            else:
                nc.scalar.copy(conn_e, conn_ps)

            # Qpacked[c, j*8+h] = qTq[c, g*16+j] * Mask[c, j*8+h]
            qpack = qpackp.tile([P, P], BF16)
            qsrc = qTq[:, g * G : (g + 1) * G]
            qrep = bass.AP(
                tensor=qsrc.tensor,
                offset=qsrc.offset,
                ap=[qsrc.ap[0], qsrc.ap[1], [0, H]],
            )
            nc.vector.tensor_mul(
                qpack.rearrange("p (j h) -> p j h", h=H),
                qrep,
                mask.rearrange("p (j h) -> p j h", h=H),
            )

            # logits: J + R accumulated into one PSUM bank
            Pattn = ps_attn.tile([P, N], F32, tag="attn")
            nc.tensor.matmul(Pattn, lhsT=qpack, rhs=kT, start=True, stop=False)
            # rounds r: the 4 matmuls {j = 4q+r} hit disjoint 32-col strips of
            # the PE array (tile_position) and stream concurrently.
            for r in range(4):
                for q in range(4):
                    j = q * 4 + r
                    nc.tensor.matmul(
                        Pattn[q * 32 : (q + 1) * 32, :],
                        lhsT=wrj4[:, r, :],
                        rhs=relg[:, j * N : (j + 1) * N],
                        start=False,
                        stop=(r == 3),
                        tile_position=(0, q * 32),
                        skip_group_check=True,
                    )
            return Pattn, conn_e

        def phase_softmax(g, Pattn, conn_e):
            logits = logitp.tile([P, N], BF16)
            nc.vector.tensor_mul(logits, Pattn, conn_e)
            attn_w = attnwp.tile([P, N], BF16)
            sums = smallp.tile([P, 1], F32)
            nc.scalar.activation(
                out=attn_w, in_=logits,
                func=mybir.ActivationFunctionType.Exp,
                scale=SCALE, accum_out=sums,
            )
            recip = smallp.tile([P, 1], F32)
            nc.vector.reciprocal(recip, sums)
            nc.vector.tensor_scalar_mul(attn_w, attn_w, recip)
            return attn_w

        def phase_tail(g, attn_w):
            # aT[m_local, (chunk, j*8+h)] via PE transpose + ACT copy
            PT = ps_tp.tile([P, N], BF16, tag="tp")
            for c in range(MC):
                nc.tensor.transpose(
                    PT[:, c * P : (c + 1) * P], attn_w[:, c * P : (c + 1) * P], ident
                )
            aT = aTp.tile([P, N], BF16)
            # NOTE: no u32-bitcast here — that trick is DVE-only; ScalarE's
            # float datapath corrupts u32 values above 2^24.
            nc.scalar.copy(aT, PT)

            # x' = attn @ v (with cross-head garbage), masked+reduced to xs2
            PX = ps_x.tile([P, P], F32, tag="px")
            for c in range(MC):
                nc.tensor.matmul(
                    PX, lhsT=vnat[:, c, :], rhs=aT[:, c * P : (c + 1) * P],
                    start=(c == 0), stop=(c == MC - 1),
                )
            xsb = xsbp.tile([P, P], BF16)
            nc.vector.tensor_mul(xsb, PX, mask)
            xs2 = smallp.tile([P, G], F32)
            nc.vector.reduce_sum(
                xs2, xsb.rearrange("p (j h) -> p j h", h=H), axis=mybir.AxisListType.X
            )
            xs2b = smallp.tile([P, G], BF16)
            nc.vector.tensor_copy(xs2b, xs2)

            PO = ps_o.tile([G, C], F32)
            nc.tensor.matmul(PO, lhsT=xs2b, rhs=wpT, start=True, stop=False)
            nc.tensor.matmul(PO, lhsT=ones16, rhs=bp, start=False, stop=True)
            out_sb = outp.tile([G, C], F32)
            nc.scalar.copy(out_sb, PO)
            nc.scalar.dma_start(out=out_d[g * G : (g + 1) * G, :], in_=out_sb)

        for g in range(NG + 1):
            if g < NG:
                Pattn, conn_e = phase_head(g)
            if g > 0:
                phase_tail(g - 1, prev)
            if g < NG:
                prev = phase_softmax(g, Pattn, conn_e)

    return nc


_GRAPH_CACHE = {}


def _get_graph(NH):
    if NH not in _GRAPH_CACHE:
        nc = build_graph(NH)
        nc.finalize()
        _GRAPH_CACHE[NH] = nc
    return _GRAPH_CACHE[NH]


def _bf16(x):
    return np.ascontiguousarray(x.astype(ml_dtypes.bfloat16))


def make_in_maps(joint_feature, relation_feature, conn_feature, W_qkv, W_r, W_proj, b_proj):
    """Shard full inputs into 8 per-core input maps (layout/dtype prep only)."""
    NH = N // 2
    wqkvT = _bf16(W_qkv.T)  # [C_in, 3C_out]: q | k | v column sections
    wrT = _bf16(W_r.T)
    wpT = _bf16(W_proj.T)
    bp = _bf16(b_proj[None, :])
    in_maps = []
    for core in range(NCORES):
        b = core // 2
        half = core % 2
        n0 = half * NH
        jT = _bf16(joint_feature[b].T)
        jTq = _bf16(joint_feature[b, n0 : n0 + NH].T)
        relT = np.ascontiguousarray(
            relation_feature[b, n0 : n0 + NH]
            .reshape(NH * N, C)
            .T.astype(ml_dtypes.float8_e4m3)
        )
        conn = _bf16(conn_feature[b, n0 : n0 + NH])
        in_maps.append(
            {
                "relT": relT,
                "conn": conn,
                "jointT": jT,
                "jointTq": jTq,
                "wqkvT": wqkvT,
                "wrT": wrT,
                "wprojT": wpT,
                "bproj": bp,
            }
        )
    return in_maps


def kernel(joint_feature, relation_feature, conn_feature, W_qkv, W_r, W_proj, b_proj):
    from concourse.bass_utils import run_bass_kernel_spmd

    NH = N // 2
    nc = _get_graph(NH)
    in_maps = make_in_maps(
        joint_feature, relation_feature, conn_feature, W_qkv, W_r, W_proj, b_proj
    )
    res = run_bass_kernel_spmd(nc, in_maps, core_ids=list(range(NCORES)))
    out = np.zeros((B, N, C), dtype=np.float32)
    for core in range(NCORES):
        b = core // 2
        half = core % 2
        n0 = half * NH
        out[b, n0 : n0 + NH] = res.results[core]["out"]
    return out
